# revision 59
# baseline (speedup 1.0000x reference)
"""TRN2 Bass kernel for nn_GAT (gnn_message_passing).

3-layer GAT stack: per layer h = relu(x@W+b); e = lrelu(s1[i]+s2[j]) masked by
adj; x += softmax_j(e) @ h.   B=8 graphs, N=2048 nodes, D=128 features.

Sharding: data-parallel over the batch dim - one graph per NeuronCore (8
cores), tiny per-layer weights replicated to every core.

Device algorithm (per core; feature dim on partitions, node index on the free
axis). With t = s1[i] + s2[j] and lrelu(t) = 0.2t + 0.8relu(t):
    exp(lrelu(t)) = exp(0.2 s1[i]) * exp(0.2 s2[j]) * max(exp(0.8 t), 1)
The exp(0.2 s1[i]) factor cancels between softmax numerator and denominator
(no-max softmax is numerically safe: |t| < 5). Folding exp(0.2 s2[j]) into
the max gives the per-edge numerator weight

    w[j,i] = mask[j,i] * max(E1bc[:,i] * E2f[j],  e20[j])

where E1bc = exp(0.8 s1) broadcast over partitions, E2f[j] = exp(s2_j) and
e20[j] = exp(0.2 s2_j) are per-partition columns. So each 128x2048 attention
tile costs exactly two ops: a 4x-mode DVE tensor_scalar (mult + max against
two column scalars) and a 2x-mode tensor_tensor mask multiply (DVE, or
GPSIMD for a few tiles to balance) - no NxN work on ACT at all.

yT[d,i] = sum_j hh[j,d] w[j,i]  (hh = relu(h)^T, fp16 stationary)
den[i]  = sum_j w[j,i]          (all-ones fp16 stationary)
accumulate in fp32 PSUM; 1/den = exp(-ln den) on ACT; xT += yT * (1/den).
All matmuls run fp16 moving data (1 PE cycle/column vs 4 for fp32).

Host side: x -> xT fp16, adj -> 0/1 adjT fp16, weights fp16 (a1 pre-scaled by
0.8, plus a column-replicated a1 matrix so s1 broadcasts across partitions in
one matmul); output comes back transposed fp32 and is flipped on the host.
"""

import numpy as np

B, N, D, L = 8, 2048, 128, 3
NT = N // 128
NCH = N // 512


def _apply_tilefix():
    """This walrus build rejects >1 sync wait on an instruction; the stock
    Tile exit-drain carries several. Spread them across single-wait NOPs."""
    import concourse.tile as tile_mod
    from concourse import mybir

    def _patched_drain_and_barrier(self, tick_clock, wait_clock):
        from concourse.tile import ScopedClock

        drain_inst = self.nc.sync.drain()
        wait_clock.add_sem_waits(
            drain_inst.ins, ScopedClock({None: tick_clock.global_clock})
        )
        si = drain_inst.ins.sync_info
        if si is not None and len(si.on_wait) > 1:
            extra = list(si.on_wait[1:])
            del si.on_wait[1:]
            for w in extra:
                nop = self.nc.sync.nop()
                nop.ins.sync_info = mybir.SyncInfo(on_wait=[w], on_update=[])
        self.nc.all_engine_barrier()
        assert self.sems is not None
        popped = self.nc._tile_sem_poison_stack.pop()
        assert popped is self._sem_poison
        self.nc.clear_and_free_semaphores(list(self.sems.allocated().values()))
        self.nc.all_engine_barrier()

    tile_mod.TileContext._drain_and_barrier = _patched_drain_and_barrier


def _split_sync_waits(nc):
    """Hoist extra sync waits onto same-engine NOPs (walrus one-wait limit)."""
    from concourse import mybir

    n = 0
    for fn in nc.m.functions:
        for bb in fn.blocks:
            new_insts = []
            changed = False
            for inst in bb.instructions:
                si = inst.sync_info
                if si is not None and len(si.on_wait) > 1:
                    extra = list(si.on_wait[1:])
                    del si.on_wait[1:]
                    for w in extra:
                        nop = mybir.InstNoOp(name=f"waitsplit-{n}", ins=[], outs=[])
                        n += 1
                        nop.engine = inst.engine
                        nop.sync_info = mybir.SyncInfo(on_wait=[w], on_update=[])
                        new_insts.append(nop)
                    changed = True
                new_insts.append(inst)
            if changed:
                bb.instructions[:] = new_insts
    return n


def build_gat(
    reps=1, gps_late=0, att_bufs=8, act_q=(), hh_eng="dve", norm_ch=2,
    norm_recip0=False, norm_c0=None, mask_in_loop=True, pp_inplace=False,
    skip_bias=False, den_late=False, q_bufs=None, hpp_early=True, tt_pair=False,
    probe=None,
):
    """Build the Bass program. reps>1 wraps the body in a For_i (timing).

    act_q: attention tiles whose q pass runs on the (otherwise idle) ACT
    engine as Exp(s1bc + s2col-bias); the max(.,e20)*mask for those tiles is
    one fused DVE scalar_tensor_tensor. Other tiles: DVE tensor_scalar +
    tensor_tensor. gps_tiles moves the mask multiply of chosen tiles to
    GPSIMD (measured slow on HW - leave empty).

    probe: None for the real kernel; "pe_only" replaces the per-tile DVE
    attention elementwise with one constant pp tile (wrong results, times the
    PE stream); "no_den" drops the denominator matmuls; "no_mm" drops the
    attention matmuls (times elementwise+prep).
    """
    import contextlib

    import concourse.bass as bass
    import concourse.tile as tile
    from concourse import mybir

    f32 = mybir.dt.float32
    f16 = mybir.dt.float16
    A = mybir.AluOpType
    F = mybir.ActivationFunctionType

    nc = bass.Bass("TRN2", target_bir_lowering=False, debug=False, num_devices=8)

    xt_in = nc.dram_tensor("xt_in", [128, N], f16, kind="ExternalInput").ap()
    maskt = nc.dram_tensor("maskt", [N, N], f16, kind="ExternalInput").ap()
    wg = nc.dram_tensor("wg", [L, D, D], f16, kind="ExternalInput").ap()
    bgc = nc.dram_tensor("bgc", [L, D], f32, kind="ExternalInput").ap()
    bgr = nc.dram_tensor("bgr", [L, D], f16, kind="ExternalInput").ap()
    a12 = nc.dram_tensor("a12", [L, D, 2], f16, kind="ExternalInput").ap()
    a1m = nc.dram_tensor("a1m", [L, D, D], f16, kind="ExternalInput").ap()
    xt_out = nc.dram_tensor("xt_out", [128, N], f32, kind="ExternalOutput").ap()

    with tile.TileContext(nc) as tc:
        ctx = contextlib.ExitStack()
        with ctx:
            consts = ctx.enter_context(tc.tile_pool(name="consts", bufs=1))
            mask_pool = ctx.enter_context(tc.tile_pool(name="mask", bufs=1))
            xt_pool = ctx.enter_context(tc.tile_pool(name="xt", bufs=2))
            hT_pool = ctx.enter_context(tc.tile_pool(name="hT", bufs=1))
            hh_pool = ctx.enter_context(tc.tile_pool(name="hh", bufs=1))
            vec_pool = ctx.enter_context(tc.tile_pool(name="vec", bufs=1))
            att_pool = ctx.enter_context(tc.tile_pool(name="att", bufs=att_bufs))
            q_pool = (
                ctx.enter_context(tc.tile_pool(name="qp", bufs=q_bufs))
                if q_bufs
                else att_pool
            )
            late_pool = ctx.enter_context(tc.tile_pool(name="late", bufs=1))
            norm_pool = ctx.enter_context(tc.tile_pool(name="norm", bufs=1))

            ones_row = consts.tile([1, 128], f16)
            nc.vector.memset(ones_row, 1.0)
            ones128 = consts.tile([128, 128], f16)
            nc.vector.memset(ones128, 1.0)
            ppc = None
            if probe == "pe_only":
                ppc = consts.tile([128, N], f16, tag="ppc")
                nc.vector.memset(ppc, 0.0005)
            Ws, b_cols, b_rows, a12cols, a1mats = [], [], [], [], []
            for l in range(L):
                W = consts.tile([128, 128], f16, tag=f"W{l}")
                nc.gpsimd.dma_start(out=W[:], in_=wg[l])
                Ws.append(W)
                bc = consts.tile([128, 1], f32, tag=f"bc{l}")
                nc.gpsimd.dma_start(
                    out=bc[:], in_=bgc[l].rearrange("(d one) -> d one", one=1)
                )
                b_cols.append(bc)
                br = consts.tile([1, 128], f16, tag=f"br{l}")
                nc.gpsimd.dma_start(
                    out=br[:], in_=bgr[l].rearrange("(one d) -> one d", one=1)
                )
                b_rows.append(br)
                ac = consts.tile([128, 2], f16, tag=f"a12{l}")
                nc.gpsimd.dma_start(out=ac[:], in_=a12[l])
                a12cols.append(ac)
                am = consts.tile([128, 128], f16, tag=f"a1m{l}")
                nc.gpsimd.dma_start(out=am[:], in_=a1m[l])
                a1mats.append(am)

            mask_hoisted = [None]
            if not mask_in_loop:
                # diagnostic: load the mask once, outside the timing loop
                mh = mask_pool.tile([128, NT * N], f16, tag="mask_h")
                mask_hoisted[0] = mh
                for jt in range(NT):
                    eng = nc.sync if jt % 2 == 0 else nc.scalar
                    eng.dma_start(
                        out=mask_hoisted[0][:, jt * N : (jt + 1) * N],
                        in_=maskt[jt * 128 : (jt + 1) * 128, :],
                    )

            def body():
                if mask_in_loop:
                    mask_sb = mask_pool.tile([128, NT * N], f16)
                else:
                    mask_sb = mask_hoisted[0]
                xT = xt_pool.tile([128, N], f16)
                nc.sync.dma_start(out=xT[:], in_=xt_in[:])
                if mask_in_loop:
                    for jt in range(NT):
                        eng = nc.sync if jt % 2 == 0 else nc.scalar
                        eng.dma_start(
                            out=mask_sb[:, jt * N : (jt + 1) * N],
                            in_=maskt[jt * 128 : (jt + 1) * 128, :],
                        )

                for l in range(L):
                    W = Ws[l]
                    prep_ctx = contextlib.ExitStack()
                    ps_big = prep_ctx.enter_context(
                        tc.tile_pool(name=f"ps_big{l}", bufs=1, space="PSUM")
                    )
                    ps_small = prep_ctx.enter_context(
                        tc.tile_pool(name=f"ps_small{l}", bufs=2, space="PSUM")
                    )
                    ps_col = prep_ctx.enter_context(
                        tc.tile_pool(name=f"ps_col{l}", bufs=1, space="PSUM")
                    )
                    # hT = relu(W.T @ xT + b); then s1bc = a1mat.T @ hT and
                    # E1bc = exp(s1bc), full-tile ACT ops (few, coarse ops -
                    # HW per-instruction overhead beats chunked pipelining)
                    hT = hT_pool.tile([128, N], f16)
                    E1bc = vec_pool.tile([128, N], f16, tag="E1bc")
                    hT_ps = ps_big.tile([128, N], f32, tag="big")
                    for c in range(NCH):
                        sl = slice(c * 512, (c + 1) * 512)
                        nc.tensor.matmul(hT_ps[:, sl], W[:], xT[:, sl])
                    nc.scalar.activation(
                        hT[:], hT_ps[:], F.Relu, bias=b_cols[l][:], scale=1.0
                    )
                    hh = hh_pool.tile([128, NT * 128], f16, tag="hh")

                    def emit_hpp(hh=hh, W=W, l=l):
                        # hh = relu(h)^T via PE recompute in [j, d] layout
                        for jt in range(NT):
                            sl = slice(jt * 128, (jt + 1) * 128)
                            hpp_ps = ps_small.tile([128, 128], f32, tag="small")
                            if skip_bias:
                                nc.tensor.matmul(
                                    hpp_ps[:], xT[:, sl], W[:], start=True, stop=True
                                )
                            else:
                                nc.tensor.matmul(
                                    hpp_ps[:], xT[:, sl], W[:], start=True, stop=False
                                )
                                nc.tensor.matmul(
                                    hpp_ps[:], ones_row[:], b_rows[l][:],
                                    start=False, stop=True,
                                )
                            if hh_eng == "act":
                                nc.scalar.activation(hh[:, sl], hpp_ps[:], F.Relu)
                            else:
                                nc.vector.tensor_scalar(
                                    hh[:, sl], hpp_ps[:], 0.0, None, A.max
                                )

                    if hpp_early:
                        # fills the PE gap while ACT runs the hT relu
                        emit_hpp()
                    s1bc_ps = ps_big.tile([128, N], f32, tag="big")
                    for c in range(NCH):
                        sl = slice(c * 512, (c + 1) * 512)
                        nc.tensor.matmul(s1bc_ps[:, sl], a1mats[l][:], hT[:, sl])
                    nc.scalar.activation(E1bc[:], s1bc_ps[:], F.Exp, scale=1.0)
                    s1bc_sb = None
                    if act_q:
                        s1bc_sb = vec_pool.tile([128, N], f32, tag="s1bc_sb")
                        nc.scalar.activation(s1bc_sb[:], s1bc_ps[:], F.Copy)
                    # s2 as per-partition columns -> E2f = exp(s2), e20 = exp(0.2 s2)
                    scols_ps = ps_col.tile([128, NT], f32, tag="scols")
                    for jt in range(NT):
                        nc.tensor.matmul(
                            scols_ps[:, jt : jt + 1],
                            hT[:, jt * 128 : (jt + 1) * 128],
                            a12cols[l][:, 1:2],
                        )
                    E2f = vec_pool.tile([128, NT], f32, tag="E2f")
                    nc.scalar.activation(E2f[:], scols_ps[:], F.Exp, scale=1.0)
                    e20 = vec_pool.tile([128, NT], f32, tag="e20")
                    nc.scalar.activation(e20[:], scols_ps[:], F.Exp, scale=0.2)
                    scols_sb = None
                    if act_q:
                        scols_sb = vec_pool.tile([128, NT], f32, tag="scols_sb")
                        nc.scalar.activation(scols_sb[:], scols_ps[:], F.Copy)
                    if not hpp_early:
                        emit_hpp()
                    prep_ctx.close()
                    # attention: w = min(max(E1bc*E2f[j], e20[j]), mask)
                    attn_ctx = contextlib.ExitStack()
                    ps_y = attn_ctx.enter_context(
                        tc.tile_pool(name=f"ps_y{l}", bufs=1, space="PSUM")
                    )
                    ps_d = attn_ctx.enter_context(
                        tc.tile_pool(name=f"ps_d{l}", bufs=1, space="PSUM")
                    )
                    yT_ps = ps_y.tile([128, N], f32, tag="y")
                    den_ps = ps_d.tile([128, N], f32, tag="d")
                    # the last gps_late tiles' mask multiplies run on the
                    # otherwise-idle GPSIMD, issued at attention start so its
                    # low throughput is hidden; this keeps DVE strictly
                    # faster than the PE so the matmul stream never starves
                    # (starvation resets the PE clock ramp)
                    pps = []
                    late = {}
                    if probe is None:
                        for jt in range(NT - gps_late, NT):
                            q = late_pool.tile([128, N], f16, tag="ql")
                            nc.vector.tensor_scalar(
                                q[:], E1bc[:], E2f[:, jt : jt + 1],
                                e20[:, jt : jt + 1], A.mult, A.max,
                            )
                            pp = late_pool.tile([128, N], f16, tag=f"pl{jt}")
                            nc.gpsimd.tensor_tensor(
                                pp[:], q[:], mask_sb[:, jt * N : (jt + 1) * N],
                                A.mult,
                            )
                            late[jt] = pp
                    if tt_pair and probe is None and not act_q and not late:
                        # pair adjacent tiles: two 4x tensor_scalar q passes
                        # into one buffer, ONE 2x mask multiply over both
                        # (mask tiles are contiguous in SBUF) - halves the
                        # DVE op count in the attention hot loop
                        for jp in range(NT // 2):
                            j0 = 2 * jp
                            q2 = q_pool.tile([128, 2 * N], f16, tag="q")
                            for k in range(2):
                                jt = j0 + k
                                nc.vector.tensor_scalar(
                                    q2[:, k * N : (k + 1) * N], E1bc[:],
                                    E2f[:, jt : jt + 1], e20[:, jt : jt + 1],
                                    A.mult, A.max,
                                )
                            pp2 = att_pool.tile([128, 2 * N], f16, tag="pp")
                            nc.vector.tensor_tensor(
                                pp2[:], q2[:],
                                mask_sb[:, j0 * N : (j0 + 2) * N], A.mult,
                            )
                            for k in range(2):
                                jt = j0 + k
                                hsl = slice(jt * 128, (jt + 1) * 128)
                                groups = (
                                    [("y",), ("den",)]
                                    if jt < NT - 1
                                    else [("den",), ("y",)]
                                )
                                for which, in groups:
                                    tgt, stat = (
                                        (yT_ps, hh[:, hsl])
                                        if which == "y"
                                        else (den_ps, ones128[:])
                                    )
                                    for c in range(NCH):
                                        sl = slice(c * 512, (c + 1) * 512)
                                        psl = slice(
                                            k * N + c * 512, k * N + (c + 1) * 512
                                        )
                                        nc.tensor.matmul(
                                            tgt[:, sl], stat, pp2[:, psl],
                                            start=(jt == 0), stop=(jt == NT - 1),
                                        )
                        tiles_iter = []
                    else:
                        tiles_iter = list(range(NT))
                    for jt in tiles_iter:
                        if probe == "pe_only":
                            pp = ppc
                        elif jt in late:
                            pp = late[jt]
                        elif jt in act_q:
                            # q on ACT; fused (max e20)*mask on DVE
                            q = att_pool.tile([128, N], f16, tag="q")
                            nc.scalar.activation(
                                q[:], s1bc_sb[:], F.Exp,
                                bias=scols_sb[:, jt : jt + 1], scale=1.0,
                            )
                            pp = att_pool.tile([128, N], f16, tag="pp")
                            nc.vector.scalar_tensor_tensor(
                                pp[:], q[:], e20[:, jt : jt + 1],
                                mask_sb[:, jt * N : (jt + 1) * N], A.max, A.mult,
                            )
                        else:
                            q = q_pool.tile([128, N], f16, tag="q")
                            nc.vector.tensor_scalar(
                                q[:], E1bc[:], E2f[:, jt : jt + 1], e20[:, jt : jt + 1],
                                A.mult, A.max,
                            )
                            if pp_inplace:
                                pp = q
                            else:
                                pp = att_pool.tile([128, N], f16, tag="pp")
                            nc.vector.tensor_tensor(
                                pp[:], q[:], mask_sb[:, jt * N : (jt + 1) * N], A.mult
                            )
                        if probe == "no_mm":
                            continue
                        hsl = slice(jt * 128, (jt + 1) * 128)
                        if den_late:
                            pps.append(pp)
                            for c in range(NCH):
                                sl = slice(c * 512, (c + 1) * 512)
                                nc.tensor.matmul(
                                    yT_ps[:, sl], hh[:, hsl], pp[:, sl],
                                    start=(jt == 0), stop=(jt == NT - 1),
                                )
                            continue
                        # den before y on the last tile so the norm chain
                        # (which waits on den) starts one matmul-group sooner
                        groups = [("y",), ("den",)] if jt < NT - 1 else [("den",), ("y",)]
                        for which, in groups:
                            if which == "den" and probe == "no_den":
                                continue
                            tgt, stat = (
                                (yT_ps, hh[:, hsl]) if which == "y" else (den_ps, ones128[:])
                            )
                            for c in range(NCH):
                                sl = slice(c * 512, (c + 1) * 512)
                                nc.tensor.matmul(
                                    tgt[:, sl], stat, pp[:, sl],
                                    start=(jt == 0), stop=(jt == NT - 1),
                                )
                    if den_late and probe != "no_den":
                        for jt in range(NT):
                            for c in range(NCH):
                                sl = slice(c * 512, (c + 1) * 512)
                                nc.tensor.matmul(
                                    den_ps[:, sl], ones128[:], pps[jt][:, sl],
                                    start=(jt == 0), stop=(jt == NT - 1),
                                )
                    # 1/den = exp(-ln den); xT_new = xT + yT/den, chunked so
                    # the next layer's hT matmuls start before the full norm
                    last = l == L - 1
                    xT_new = xt_pool.tile([128, N], f32 if last else f16)
                    if probe == "no_mm":
                        nc.vector.tensor_tensor(xT_new[:], E1bc[:], xT[:], A.add)
                    else:
                        d_src = yT_ps if probe == "no_den" else den_ps
                        r = norm_pool.tile([128, N], f32, tag="nB")
                        lnd = norm_pool.tile([128, N], f32, tag="nA")
                        yt = norm_pool.tile([128, N], f32 if last else f16, tag="nC")
                        if norm_c0 is not None:
                            bounds = [0, norm_c0, N]
                        else:
                            cw = N // norm_ch
                            bounds = [i * cw for i in range(norm_ch)] + [N]
                        for c in range(len(bounds) - 1):
                            sl = slice(bounds[c], bounds[c + 1])
                            if c == 0 and norm_recip0:
                                # critical first chunk: low-latency DVE divide
                                nc.vector.reciprocal(r[:, sl], d_src[:, sl])
                            else:
                                # 1/den = exp(-ln den) on the (idle) ACT engine
                                nc.scalar.activation(lnd[:, sl], d_src[:, sl], F.Ln)
                                nc.scalar.activation(
                                    r[:, sl], lnd[:, sl], F.Exp, scale=-1.0
                                )
                            nc.vector.tensor_tensor(yt[:, sl], yT_ps[:, sl], r[:, sl], A.mult)
                            nc.vector.tensor_tensor(xT_new[:, sl], yt[:, sl], xT[:, sl], A.add)
                    attn_ctx.close()
                    xT = xT_new

                nc.sync.dma_start(out=xt_out[:], in_=xT[:])

            if reps == 1:
                body()
            else:
                with tc.For_i(0, reps, 1):
                    body()

    return nc


def host_prep(x, adj, Wg, bg, attn_a):
    a12 = np.stack([0.8 * attn_a[:, :D], attn_a[:, D:]], axis=2)  # [L, D, 2]
    # a1m[l, d, p] = 0.8*a1[l, d] for all p: stationary that broadcasts
    # s1 = h @ (0.8 a1) across all 128 output partitions in one matmul
    a1m = np.repeat(0.8 * attn_a[:, :D, None], D, axis=2)  # [L, D, D]
    in_maps = []
    for b in range(B):
        in_maps.append(
            {
                "xt_in": np.ascontiguousarray(x[b].T).astype(np.float16),
                "maskt": (np.ascontiguousarray(adj[b].T) > 0).astype(np.float16),
                "wg": np.ascontiguousarray(Wg, np.float16),
                "bgc": np.ascontiguousarray(bg, np.float32),
                "bgr": np.ascontiguousarray(bg, np.float16),
                "a12": np.ascontiguousarray(a12, np.float16),
                "a1m": np.ascontiguousarray(a1m, np.float16),
            }
        )
    return in_maps


def host_post(results):
    return np.stack([results[b]["xt_out"].T for b in range(B)]).astype(np.float32)


def kernel(x, adj, Wg, bg, attn_a):
    x = np.asarray(x)
    adj = np.asarray(adj)
    Wg = np.asarray(Wg)
    bg = np.asarray(bg)
    attn_a = np.asarray(attn_a)

    _apply_tilefix()
    from concourse.bass_utils import run_bass_kernel_spmd

    nc = build_gat(reps=1)
    _split_sync_waits(nc)
    in_maps = host_prep(x, adj, Wg, bg, attn_a)
    res = run_bass_kernel_spmd(nc, in_maps, core_ids=list(range(B)))
    return host_post(res.results)
